# revision 15
# baseline (speedup 1.0000x reference)
"""EMS loss (margin-scaled cross-entropy, mean reduction) on 8 TRN2 NeuronCores.

Full inputs -> shard batch dim 8-way (512 rows/core) -> per-core Bass kernel:
  S[row]  = sum_j exp(x[row, j])            (streamed, ACT exp + fused row-sum)
  v[row]  = x[row, target[row]]             (indirect-DMA gather)
  nll     = log(S - exp(v) + exp(4 v)) - 4 v
  loss    = AllReduce_sum(sum(nll) / 4096) over the 8 cores
No max-subtraction: inputs are bounded (randn; |4 v| <= ~25), exp stays in f32
range and ACT exp is ~2 ULP.
"""

import os
import sys

sys.path.insert(0, "/opt/trn_rl_repo")

import numpy as np

import concourse.bacc as bacc
import concourse.bass as bass
import concourse.mybir as mybir
import concourse.tile as tile
from concourse.bass_utils import run_bass_kernel_spmd

N_CORES = 8
B = 4096            # global batch
V = 32000           # vocab
RPC = B // N_CORES  # rows per core = 512
P = 128             # SBUF partitions
RB = RPC // P       # row blocks per core = 4
F = 8000            # free-dim chunk
NCH = V // F        # chunks per row = 4
MARGIN = 4.0
XBUFS = 5           # streaming tile pool depth
ALT_DMA = False     # alternate sync/scalar HWDGE rings for streaming DMAs
ACT_SCRATCH = 0     # 0: exp in-place over xt; N: write exp to scratch pool bufs=N

_cache = {}


def _build(repeats=1, tail_every_rep=True):
    """Build the per-core Bass program. `repeats` unrolls the whole body N
    times (same math, same output) — used only for delta-timing on HW.
    `tail_every_rep=False` runs the gather/correction/collective tail only on
    the last rep (bench-only; isolates pure streaming marginal)."""
    nc = bacc.Bacc(
        "TRN2",
        target_bir_lowering=False,
        debug=False,
        num_devices=N_CORES,
    )
    f32 = mybir.dt.float32
    i32 = mybir.dt.int32

    x = nc.dram_tensor("inputs", [RPC, V], f32, kind="ExternalInput").ap()
    tgt = nc.dram_tensor("targets", [P, RB], i32, kind="ExternalInput").ap()
    out = nc.dram_tensor("out", [1, 1], f32, kind="ExternalOutput").ap()
    cc_in = nc.dram_tensor("cc_in", [1, 1], f32).ap()
    cc_out = nc.dram_tensor("cc_out", [1, 1], f32).ap()

    with tile.TileContext(nc) as tc:
        with (
            tc.tile_pool(name="xp", bufs=XBUFS) as xp,
            tc.tile_pool(name="scr", bufs=max(ACT_SCRATCH, 1)) as scr,
            tc.tile_pool(name="small", bufs=1) as small,
            tc.tile_pool(name="ps", bufs=1, space="PSUM") as ps,
        ):
          for _rep in range(repeats):
           is_last = _rep == repeats - 1
           if tail_every_rep or is_last:
            # ---- target-logit gather: idx = row*V + target (flat element idx)
            tgt_s = small.tile([P, RB], i32)
            nc.sync.dma_start(out=tgt_s[:], in_=tgt)
            base = small.tile([P, RB], i32)
            # base[p, r] = r*128 + p  == local row  (iota step must fit int16)
            nc.gpsimd.iota(base[:], pattern=[[P, RB]], base=0, channel_multiplier=1)
            # base *= V  -> flat element offset of row start
            nc.gpsimd.tensor_scalar(
                out=base[:],
                in0=base[:],
                scalar1=V,
                scalar2=None,
                op0=mybir.AluOpType.mult,
            )
            idx = small.tile([P, RB], i32)
            nc.gpsimd.tensor_tensor(
                out=idx[:], in0=tgt_s[:], in1=base[:], op=mybir.AluOpType.add
            )

            # flat view of x with per-dim counts < 2^16
            x_flat = x.rearrange("a (b c) -> (a b) c", c=1000)
            v = small.tile([P, RB], f32)
            for r in range(RB):
                nc.gpsimd.indirect_dma_start(
                    out=v[:, r : r + 1],
                    out_offset=None,
                    in_=x_flat,
                    in_offset=bass.IndirectOffsetOnAxis(
                        ap=idx[:, r : r + 1], axis=1
                    ),
                )

           # ---- streaming sum-of-exp: acc[p, r*NCH+c] = sum_f exp(x_tile)
           if True:
            acc = small.tile([P, RB * NCH], f32)
            for r in range(RB):
                for c in range(NCH):
                    xt = xp.tile([P, F], f32, tag="xt")
                    k = r * NCH + c
                    dma_eng = (
                        (nc.sync, nc.scalar)[k % 2] if ALT_DMA else nc.sync
                    )
                    dma_eng.dma_start(
                        out=xt[:], in_=x[r * P : (r + 1) * P, c * F : (c + 1) * F]
                    )
                    if ACT_SCRATCH:
                        et = scr.tile([P, F], f32, tag="et")
                        nc.scalar.activation(
                            out=et[:],
                            in_=xt[:],
                            func=mybir.ActivationFunctionType.Exp,
                            accum_out=acc[:, k : k + 1],
                        )
                    else:
                        nc.scalar.activation(
                            out=xt[:],
                            in_=xt[:],
                            func=mybir.ActivationFunctionType.Exp,
                            accum_out=acc[:, k : k + 1],
                        )

           # ---- tail: per-row denominator, margin correction, nll
           if tail_every_rep or is_last:
            s = small.tile([P, RB], f32)
            nc.vector.reduce_sum(
                out=s[:],
                in_=acc[:].rearrange("p (r c) -> p r c", r=RB),
                axis=mybir.AxisListType.X,
            )
            ev = small.tile([P, RB], f32)
            nc.scalar.activation(
                out=ev[:], in_=v[:], func=mybir.ActivationFunctionType.Exp
            )
            e4 = small.tile([P, RB], f32)
            nc.scalar.activation(
                out=e4[:],
                in_=v[:],
                func=mybir.ActivationFunctionType.Exp,
                scale=MARGIN,
            )
            sp = small.tile([P, RB], f32)
            nc.vector.tensor_tensor(
                out=sp[:], in0=s[:], in1=ev[:], op=mybir.AluOpType.subtract
            )
            nc.vector.tensor_tensor(
                out=sp[:], in0=sp[:], in1=e4[:], op=mybir.AluOpType.add
            )
            lg = small.tile([P, RB], f32)
            nc.scalar.activation(
                out=lg[:], in_=sp[:], func=mybir.ActivationFunctionType.Ln
            )
            w4 = small.tile([P, RB], f32)
            nc.vector.tensor_scalar_mul(w4[:], v[:], MARGIN)
            nll = small.tile([P, RB], f32)
            nc.vector.tensor_tensor(
                out=nll[:], in0=lg[:], in1=w4[:], op=mybir.AluOpType.subtract
            )
            rs = small.tile([P, 1], f32)
            nc.vector.reduce_sum(out=rs[:], in_=nll[:], axis=mybir.AxisListType.X)

            # partition-axis reduction via matmul with ones: [1,128] @ [128,1]
            ones = small.tile([P, 1], f32)
            nc.vector.memset(ones[:], 1.0)
            pt = ps.tile([1, 1], f32)
            nc.tensor.matmul(out=pt[:], lhsT=rs[:], rhs=ones[:], start=True, stop=True)
            res = small.tile([1, 1], f32)
            nc.scalar.mul(out=res[:], in_=pt[:], mul=1.0 / B)

            # ---- cross-core mean via AllReduce over DRAM bounce buffers
            nc.sync.dma_start(out=cc_in, in_=res[:])
            nc.gpsimd.collective_compute(
                "AllReduce",
                mybir.AluOpType.add,
                replica_groups=[list(range(N_CORES))],
                ins=[cc_in.opt()],
                outs=[cc_out.opt()],
            )
            nc.gpsimd.dma_start(out=out, in_=cc_out)

    # Pre-place one ACT table load of a set containing Exp AND Ln (e.g.
    # natural_log_exp_and_others). Otherwise the auto-pass loads
    # exp_and_others for the streaming exps and switches to natural_log for
    # the tail Ln (~2.7us per switch). The insert_act_table_loads pass
    # tracks this pre-placed load and adds nothing; if set resolution fails
    # the auto-pass still inserts correct loads on its own.
    try:
        from concourse.hw_specs import get_activation_tables

        tables = get_activation_tables(nc.m.arch)
        need = {
            mybir.ActivationFunctionType.Exp,
            mybir.ActivationFunctionType.Ln,
            mybir.ActivationFunctionType.Copy,
        }
        set_id = next(
            i for i, funcs in enumerate(tables.values()) if need <= funcs
        )
        inst = mybir.InstLoadActFuncSet(
            name=nc.get_next_instruction_name(),
            act_func_set_id=set_id,
            ins=[],
            outs=[],
        )
        inst.engine = mybir.EngineType.Activation
        nc.register_instruction(inst)
        nc.main_func.blocks[0].instructions.insert(0, inst)
    except (ImportError, StopIteration):
        pass

    nc.compile()
    return nc


def kernel(**inputs):
    x = np.ascontiguousarray(inputs["inputs"], dtype=np.float32)
    t = np.asarray(inputs["targets"])
    assert x.shape == (B, V), x.shape

    if "nc" not in _cache:
        _cache["nc"] = _build()
    nc = _cache["nc"]

    in_maps = []
    for i in range(N_CORES):
        xs = x[i * RPC : (i + 1) * RPC]
        # [P, RB] layout: partition p, col r  ->  local row r*128 + p
        ts = np.ascontiguousarray(
            t[i * RPC : (i + 1) * RPC].astype(np.int32).reshape(RB, P).T
        )
        in_maps.append({"inputs": xs, "targets": ts})

    results = run_bass_kernel_spmd(
        nc,
        in_maps,
        core_ids=list(range(N_CORES)),
        trace=bool(int(os.environ.get("EMS_TRACE", "0"))),
    )
    _cache["last_results"] = results
    return np.asarray(results.results[0]["out"][0, 0], dtype=np.float32)


# revision 20
# speedup vs baseline: 1.1280x; 1.1280x over previous
"""EMS loss (margin-scaled cross-entropy, mean reduction) on 8 TRN2 NeuronCores.

Full inputs -> shard batch dim 8-way (512 rows/core) -> per-core Bass kernel:
  S[row]  = sum_j exp(x[row, j])            (streamed, ACT exp + fused row-sum)
  v[row]  = x[row, target[row]]             (indirect-DMA gather)
  nll     = log(S - exp(v) + exp(4 v)) - 4 v
  loss    = AllReduce_sum(sum(nll) / 4096) over the 8 cores
No max-subtraction: inputs are bounded (randn; |4 v| <= ~25), exp stays in f32
range and ACT exp is ~2 ULP.
"""

import os
import sys

sys.path.insert(0, "/opt/trn_rl_repo")

import numpy as np

import concourse.bacc as bacc
import concourse.bass as bass
import concourse.mybir as mybir
import concourse.tile as tile
from concourse.bass_utils import run_bass_kernel_spmd

N_CORES = 8
B = 4096            # global batch
V = 32000           # vocab
RPC = B // N_CORES  # rows per core = 512
P = 128             # SBUF partitions
RB = RPC // P       # row blocks per core = 4
F = 8000            # free-dim chunk
NCH = V // F        # chunks per row = 4
MARGIN = 4.0
XBUFS = 5           # streaming tile pool depth
ALT_DMA = False     # alternate sync/scalar HWDGE rings for streaming DMAs
ACT_SCRATCH = 0     # 0: exp in-place over xt; N: write exp to scratch pool bufs=N
HOL_DEP = True      # pin tail exps behind streaming exps on ACT (see below)

_cache = {}


def _build(repeats=1, tail_every_rep=True):
    """Build the per-core Bass program. `repeats` unrolls the whole body N
    times (same math, same output) — used only for delta-timing on HW.
    `tail_every_rep=False` runs the gather/correction/collective tail only on
    the last rep (bench-only; isolates pure streaming marginal)."""
    nc = bacc.Bacc(
        "TRN2",
        target_bir_lowering=False,
        debug=False,
        num_devices=N_CORES,
    )
    f32 = mybir.dt.float32
    i32 = mybir.dt.int32

    x = nc.dram_tensor("inputs", [RPC, V], f32, kind="ExternalInput").ap()
    tgt = nc.dram_tensor("targets", [P, RB], i32, kind="ExternalInput").ap()
    out = nc.dram_tensor("out", [1, 1], f32, kind="ExternalOutput").ap()
    cc_in = nc.dram_tensor("cc_in", [1, 1], f32).ap()
    cc_out = nc.dram_tensor("cc_out", [1, 1], f32).ap()

    with tile.TileContext(nc) as tc:
        with (
            tc.tile_pool(name="xp", bufs=XBUFS) as xp,
            tc.tile_pool(name="scr", bufs=max(ACT_SCRATCH, 1)) as scr,
            tc.tile_pool(name="small", bufs=1) as small,
            tc.tile_pool(name="ps", bufs=1, space="PSUM") as ps,
        ):
          for _rep in range(repeats):
           is_last = _rep == repeats - 1
           if tail_every_rep or is_last:
            # ---- target-logit gather: idx = row*V + target (flat element idx)
            tgt_s = small.tile([P, RB], i32)
            nc.sync.dma_start(out=tgt_s[:], in_=tgt)
            base = small.tile([P, RB], i32)
            # base[p, r] = r*128 + p  == local row  (iota step must fit int16)
            nc.gpsimd.iota(base[:], pattern=[[P, RB]], base=0, channel_multiplier=1)
            # base *= V  -> flat element offset of row start
            nc.gpsimd.tensor_scalar(
                out=base[:],
                in0=base[:],
                scalar1=V,
                scalar2=None,
                op0=mybir.AluOpType.mult,
            )
            idx = small.tile([P, RB], i32)
            nc.gpsimd.tensor_tensor(
                out=idx[:], in0=tgt_s[:], in1=base[:], op=mybir.AluOpType.add
            )

            # flat view of x with per-dim counts < 2^16
            x_flat = x.rearrange("a (b c) -> (a b) c", c=1000)
            v = small.tile([P, RB], f32)
            for r in range(RB):
                nc.gpsimd.indirect_dma_start(
                    out=v[:, r : r + 1],
                    out_offset=None,
                    in_=x_flat,
                    in_offset=bass.IndirectOffsetOnAxis(
                        ap=idx[:, r : r + 1], axis=1
                    ),
                )

           # ---- streaming sum-of-exp: acc[p, r*NCH+c] = sum_f exp(x_tile)
           if True:
            acc = small.tile([P, RB * NCH], f32)
            last_stream_act = None
            for r in range(RB):
                for c in range(NCH):
                    xt = xp.tile([P, F], f32, tag="xt")
                    k = r * NCH + c
                    dma_eng = (
                        (nc.sync, nc.scalar)[k % 2] if ALT_DMA else nc.sync
                    )
                    dma_eng.dma_start(
                        out=xt[:], in_=x[r * P : (r + 1) * P, c * F : (c + 1) * F]
                    )
                    if ACT_SCRATCH:
                        et = scr.tile([P, F], f32, tag="et")
                        last_stream_act = nc.scalar.activation(
                            out=et[:],
                            in_=xt[:],
                            func=mybir.ActivationFunctionType.Exp,
                            accum_out=acc[:, k : k + 1],
                        )
                    else:
                        last_stream_act = nc.scalar.activation(
                            out=xt[:],
                            in_=xt[:],
                            func=mybir.ActivationFunctionType.Exp,
                            accum_out=acc[:, k : k + 1],
                        )

           # ---- tail: per-row denominator, margin correction, nll
           if tail_every_rep or is_last:
            s = small.tile([P, RB], f32)
            nc.vector.reduce_sum(
                out=s[:],
                in_=acc[:].rearrange("p (r c) -> p r c", r=RB),
                axis=mybir.AxisListType.X,
            )
            ev = small.tile([P, RB], f32)
            ev_inst = nc.scalar.activation(
                out=ev[:], in_=v[:], func=mybir.ActivationFunctionType.Exp
            )
            e4 = small.tile([P, RB], f32)
            e4_inst = nc.scalar.activation(
                out=e4[:],
                in_=v[:],
                func=mybir.ActivationFunctionType.Exp,
                scale=MARGIN,
            )
            # Keep the tail exps BEHIND the streaming exps in the ACT engine
            # program: they transitively wait on the indirect-DMA gather, and
            # if the scheduler hoists them first they head-of-line-block every
            # streaming activation (and then the DMA pipeline once the tile
            # pool fills). Ordering-only dep: same engine, no semaphore.
            if HOL_DEP:
                from concourse.tile import add_dep_helper

                for inst in (ev_inst, e4_inst):
                    add_dep_helper(
                        inst.ins,
                        last_stream_act.ins,
                        sync=False,
                        reason="tail exps after streaming exps (avoid ACT HoL block)",
                    )
            sp = small.tile([P, RB], f32)
            nc.vector.tensor_tensor(
                out=sp[:], in0=s[:], in1=ev[:], op=mybir.AluOpType.subtract
            )
            nc.vector.tensor_tensor(
                out=sp[:], in0=sp[:], in1=e4[:], op=mybir.AluOpType.add
            )
            lg = small.tile([P, RB], f32)
            nc.scalar.activation(
                out=lg[:], in_=sp[:], func=mybir.ActivationFunctionType.Ln
            )
            w4 = small.tile([P, RB], f32)
            nc.vector.tensor_scalar_mul(w4[:], v[:], MARGIN)
            nll = small.tile([P, RB], f32)
            nc.vector.tensor_tensor(
                out=nll[:], in0=lg[:], in1=w4[:], op=mybir.AluOpType.subtract
            )
            rs = small.tile([P, 1], f32)
            nc.vector.reduce_sum(out=rs[:], in_=nll[:], axis=mybir.AxisListType.X)

            # partition-axis reduction via matmul with ones: [1,128] @ [128,1]
            ones = small.tile([P, 1], f32)
            nc.vector.memset(ones[:], 1.0)
            pt = ps.tile([1, 1], f32)
            nc.tensor.matmul(out=pt[:], lhsT=rs[:], rhs=ones[:], start=True, stop=True)
            res = small.tile([1, 1], f32)
            nc.scalar.mul(out=res[:], in_=pt[:], mul=1.0 / B)

            # ---- cross-core mean via AllReduce over DRAM bounce buffers
            nc.sync.dma_start(out=cc_in, in_=res[:])
            nc.gpsimd.collective_compute(
                "AllReduce",
                mybir.AluOpType.add,
                replica_groups=[list(range(N_CORES))],
                ins=[cc_in.opt()],
                outs=[cc_out.opt()],
            )
            nc.gpsimd.dma_start(out=out, in_=cc_out)

    # Pre-place one ACT table load of a set containing Exp AND Ln (e.g.
    # natural_log_exp_and_others). Otherwise the auto-pass loads
    # exp_and_others for the streaming exps and switches to natural_log for
    # the tail Ln (~2.7us per switch). The insert_act_table_loads pass
    # tracks this pre-placed load and adds nothing; if set resolution fails
    # the auto-pass still inserts correct loads on its own.
    try:
        from concourse.hw_specs import get_activation_tables

        tables = get_activation_tables(nc.m.arch)
        need = {
            mybir.ActivationFunctionType.Exp,
            mybir.ActivationFunctionType.Ln,
            mybir.ActivationFunctionType.Copy,
        }
        set_id = next(
            i for i, funcs in enumerate(tables.values()) if need <= funcs
        )
        inst = mybir.InstLoadActFuncSet(
            name=nc.get_next_instruction_name(),
            act_func_set_id=set_id,
            ins=[],
            outs=[],
        )
        inst.engine = mybir.EngineType.Activation
        nc.register_instruction(inst)
        nc.main_func.blocks[0].instructions.insert(0, inst)
    except (ImportError, StopIteration):
        pass

    nc.compile()
    return nc


def kernel(**inputs):
    x = np.ascontiguousarray(inputs["inputs"], dtype=np.float32)
    t = np.asarray(inputs["targets"])
    assert x.shape == (B, V), x.shape

    if "nc" not in _cache:
        _cache["nc"] = _build()
    nc = _cache["nc"]

    in_maps = []
    for i in range(N_CORES):
        xs = x[i * RPC : (i + 1) * RPC]
        # [P, RB] layout: partition p, col r  ->  local row r*128 + p
        ts = np.ascontiguousarray(
            t[i * RPC : (i + 1) * RPC].astype(np.int32).reshape(RB, P).T
        )
        in_maps.append({"inputs": xs, "targets": ts})

    results = run_bass_kernel_spmd(
        nc,
        in_maps,
        core_ids=list(range(N_CORES)),
        trace=bool(int(os.environ.get("EMS_TRACE", "0"))),
    )
    _cache["last_results"] = results
    return np.asarray(results.results[0]["out"][0, 0], dtype=np.float32)
